# revision 25
# baseline (speedup 1.0000x reference)
"""Trainium2 Bass kernel for nn_Fast2Order_DE_Conv (hybrid basis v4).

Math: out[b,o,ho,wo] = sum_{c,i,j} W[o, c*81+i*9+j] * p_i * p_j with
p_i = x[b, c, ho+di, wo+dj] (i = di*3+dj, 3x3 unfold of a 16-channel 64x64
image; output 62x62).

Quadratic features are evaluated in a HYBRID basis that splits work across
every engine (per channel, pairs keyed by index distance d = j - i):
  d in {1,2,3}: direct products x_i * x_j          -> DVE tensor_tensor
  d = 0, i<8  : squares x_i^2                      -> GPSIMD tensor_tensor
  d in {4..8} + x_8^2: (x_i+x_j)^2 squares basis   -> PE selection matmul
                                                      + ACT square
The basis change is folded into W on the host.  Per 512-column spatial
tile the PE runs only 2 selection + 6 main matmuls (vs 12 in a pure
squares-basis kernel); the d<=3 product features are computed at full
3968-column width, with the shifted second operands staged by SBUF->SBUF
DMA into partition-aligned pack tiles (DVE ops cannot read unaligned
partition bases).

Sharding: data-parallel over batch, 2 batches per core on 8 cores.
"""

import functools

import numpy as np

import concourse.bacc as bacc
import concourse.mybir as mybir
from concourse.tile import TileContext
from concourse.bass_utils import run_bass_kernel_spmd

B, C, H, WIDTH = 16, 16, 64, 64
O = 128
HO = WO = 62
N_CORES = 8
B_LOC = B // N_CORES
TILE_PAIRS = [((0, 8), (8, 8)), ((16, 8), (24, 8)),
              ((32, 8), (40, 8)), ((48, 8), (56, 6))]
GC = 128
SEL_PAIRS = [(i, i + d) for d in range(4, 9) for i in range(0, 9 - d)]  # 15
DEDUP_LDW = True
D0_GPS = False
ABLATE = {"hoist_packs"}
D0_SPLIT = False
X_BUFS = 1
GF_BUFS = 8
GS_BUFS = 6
O_BUFS = 2
PK_BUFS = 1


def _build_consts(Wf: np.ndarray):
    """W (128, 1296) -> (AselT [72, 128] f16, W2T [768, 128] f16).

    Per half (8 channels), 384 rows = 3 chunks of 128:
      chunk0: rows i*8+cl    (i 0..7) -> product (i, i+1)   [64]
              rows 64+i*8+cl (i 0..6) -> product (i, i+2)   [56]
      chunk1: rows i*8+cl    (i 0..7) -> x_i^2              [64]
              rows 64+i*8+cl (i 0..5) -> product (i, i+3)   [48]
      chunk2: sel (x_i+x_j)^2 for (i,j) in SEL_PAIRS [120], x_8^2 [8]
    """
    Wt = np.asarray(Wf, dtype=np.float64).reshape(O, C, 9, 9)
    Wsym = Wt + Wt.transpose(0, 1, 3, 2)
    W2 = np.zeros((O, 2, 3, 128))
    AselT = np.zeros((72, 128))
    for h in range(2):
        for cl in range(8):
            c = h * 8 + cl
            for i in range(8):
                W2[:, h, 0, i * 8 + cl] = Wsym[:, c, i, i + 1]
            for i in range(7):
                W2[:, h, 0, 64 + i * 8 + cl] = Wsym[:, c, i, i + 2]
            for i in range(8):
                W2[:, h, 1, i * 8 + cl] = Wt[:, c, i, i] - 0.5 * sum(
                    Wsym[:, c, a, b] for (a, b) in SEL_PAIRS if i in (a, b)
                )
            for i in range(6):
                W2[:, h, 1, 64 + i * 8 + cl] = Wsym[:, c, i, i + 3]
            for pi, (i, j) in enumerate(SEL_PAIRS):
                W2[:, h, 2, pi * 8 + cl] = 0.5 * Wsym[:, c, i, j]
            W2[:, h, 2, 120 + cl] = Wt[:, c, 8, 8] - 0.5 * sum(
                Wsym[:, c, a, b] for (a, b) in SEL_PAIRS if 8 in (a, b)
            )
    for cl in range(8):
        for pi, (i, j) in enumerate(SEL_PAIRS):
            AselT[i * 8 + cl, pi * 8 + cl] = 1.0
            AselT[j * 8 + cl, pi * 8 + cl] += 1.0
        AselT[64 + cl, 120 + cl] = 1.0
    W2T = np.ascontiguousarray(
        W2.transpose(1, 2, 3, 0).reshape(768, O)
    ).astype(np.float16)
    return AselT.astype(np.float16), W2T


def _x_window_ap(x_d, b: int, h: int, di: int, lt_load: int):
    """Source AP for one di of the unfold load: (dj, c, l) nesting matching
    target partitions (di*3+dj)*8 + c, free dim = padded l' = ho*64+wo."""
    ap = x_d[b, h * 8 : (h + 1) * 8, di, 0:3].unsqueeze(-1)
    v = ap.ap
    v[0] = [1, 3]
    v[1] = [H * WIDTH, 8]
    v[2] = [1, lt_load]
    return ap


def _dedup_ldweights(nc) -> int:
    """Remove InstLdweights whose weights AP matches the previous load in
    the same block (the PE array retains its stationary operand)."""
    removed = 0
    for fn in nc.m.functions:
        for blk in fn.blocks:
            insts = list(blk.instructions)
            cur_key = None
            drop = []
            for idx, inst in enumerate(insts):
                tn = type(inst).__name__
                if tn == "InstLdweights":
                    ap = inst.ins[0]
                    key = (
                        ap.memref, ap.offset, str(ap.ap), str(ap.dtype),
                        str(inst.perf_mode), str(inst.is_transpose),
                        str(inst.tile_position), str(inst.tile_size),
                    )
                    si = inst.sync_info
                    clean = si is None or (not si.on_wait and not si.on_update)
                    if key == cur_key and clean:
                        drop.append(idx)
                        removed += 1
                        continue
                    cur_key = key
                elif tn == "InstMatmult":
                    if getattr(inst, "is_transpose", None):
                        cur_key = None
            for idx in reversed(drop):
                del blk.instructions[idx]
    return removed


def build_nc(reps: int = 1, skew: int = 2, static_reps: int = 1):
    """Build the per-core program.  reps>1 wraps the body in an on-chip
    loop (For_i) for device-time measurement; skew is the pipeline depth
    in tile-pairs between selection/squares and main matmuls."""
    f32, f16 = mybir.dt.float32, mybir.dt.float16
    nc = bacc.Bacc("TRN2", target_bir_lowering=False)
    x_d = nc.dram_tensor("x_loc", [B_LOC, C, H, WIDTH], f16, kind="ExternalInput")
    a_d = nc.dram_tensor("aselT", [72, GC], f16, kind="ExternalInput")
    w_d = nc.dram_tensor("w2T", [6 * GC, O], f16, kind="ExternalInput")
    o_d = nc.dram_tensor("out_loc", [B_LOC, O, HO, WO], f32, kind="ExternalOutput")

    LFULL = HO * 64  # 3968

    with TileContext(nc) as tc:
        with (
            tc.tile_pool(name="const", bufs=1) as cpool,
            tc.tile_pool(name="xin", bufs=X_BUFS) as xpool,
            tc.tile_pool(name="packs", bufs=PK_BUFS) as packpool,
            tc.tile_pool(name="gfull", bufs=GF_BUFS) as gfpool,
            tc.tile_pool(name="gsel", bufs=GS_BUFS) as gspool,
            tc.tile_pool(name="obuf", bufs=O_BUFS) as opool,
            tc.tile_pool(name="ps_sel", bufs=2, space="PSUM") as pspool,
            tc.tile_pool(name="ps_out", bufs=2, space="PSUM") as popool,
        ):
            a_r = cpool.tile([72, GC], f16, tag="a_r")
            nc.sync.dma_start(a_r[:], a_d[:])

            def load_x(x_t, b, h, col0, col1):
                for di in range(3):
                    hi = min(col1, H * WIDTH - di * 64 - 2)
                    if hi > col0:
                        ap = _x_window_ap(x_d, b, h, di, hi - col0)
                        ap.offset += col0
                        nc.sync.dma_start(
                            x_t[di * 24 : (di + 1) * 24, col0:hi], ap
                        )
                    if hi < col1:
                        nc.sync.dma_start(
                            x_t[di * 24 : (di + 1) * 24, hi:col1],
                            _x_window_ap(x_d, b, h, 0, col1 - hi),
                        )

            xr_all = [
                [
                    xpool.tile([72, LFULL], f16, tag=f"x{b}{h}", name=f"x{b}{h}")
                    for h in range(2)
                ]
                for b in range(B_LOC)
            ]
            for h in range(2):
                load_x(xr_all[0][h], 0, h, 0, 1024)
            w_r = cpool.tile([GC, 6, O], f16, tag="w_r")
            nc.sync.dma_start(w_r[:], w_d[:].rearrange("(k p) o -> p k o", p=GC))
            for h in range(2):
                load_x(xr_all[0][h], 0, h, 1024, LFULL)
            for b in range(1, B_LOC):
                for h in range(2):
                    load_x(xr_all[b][h], b, h, 0, LFULL)

            gconst = None
            sconst = None
            if "const_g" in ABLATE or "const_selg" in ABLATE:
                if "const_g" in ABLATE:
                    gconst = [cpool.tile([GC, LFULL], f16, tag=f"gc{i}",
                                         name=f"gc{i}") for i in range(2)]
                    for t in gconst:
                        nc.vector.memset(t[:], 0.01)
                if "const_selg" in ABLATE:
                    sconst = cpool.tile([GC, 2, 512], f16, tag="sc")
                    nc.vector.memset(sconst[:], 0.01)
            if "no_mains" in ABLATE:
                o_zero = cpool.tile([O, 8 * WO], f32, tag="o_zero")
                nc.vector.memset(o_zero[:], 0.0)
                for b in range(B_LOC):
                    for ho0, nr in [(i * 8, 8) for i in range(7)] + [(56, 6)]:
                        nc.sync.dma_start(
                            o_d[b, :, ho0 : ho0 + nr, :], o_zero[:, : nr * WO]
                        )

            hoisted_packs = {}
            if "hoist_packs" in ABLATE:
                # 128-row operand packs: one DVE op per g chunk.
                # APack = [x_t[0:64]; x_t[0:64]] is the A side of both
                # chunks; BPack0/1 carry the shifted B operands (+ valid
                # junk in the zero-weight pad rows).
                for b in range(B_LOC):
                    for h in range(2):
                        x_t = xr_all[b][h]
                        pks = (
                            cpool.tile([128, LFULL], f16, tag=f"hpa{b}{h}",
                                       name=f"hpa{b}{h}"),
                            cpool.tile([128, LFULL], f16, tag=f"hpb{b}{h}",
                                       name=f"hpb{b}{h}"),
                            cpool.tile([128, LFULL], f16, tag=f"hpc{b}{h}",
                                       name=f"hpc{b}{h}"),
                        )
                        nc.sync.dma_start(pks[0][0:64, :], x_t[0:64, :])
                        nc.sync.dma_start(pks[0][64:128, :], x_t[0:64, :])
                        nc.sync.dma_start(pks[1][0:64, :], x_t[8:72, :])
                        nc.sync.dma_start(pks[1][64:120, :], x_t[16:72, :])
                        nc.sync.dma_start(pks[1][120:128, :], x_t[0:8, :])
                        nc.sync.dma_start(pks[2][0:64, :], x_t[0:64, :])
                        nc.sync.dma_start(pks[2][64:112, :], x_t[24:72, :])
                        nc.sync.dma_start(pks[2][112:128, :], x_t[0:16, :])
                        hoisted_packs[(b, h)] = pks

            def make_products(b, h):
                """Full-width product chunks 0/1 for (batch, half)."""
                x_t = xr_all[b][h]
                # base-0 staged copies of the shifted B operands (TensorTensor
                # requires equal base partitions for both SBUF inputs)
                # pk2/pk3 are widened to 64 rows with valid junk so the
                # product ops also fill the zero-weight pad rows of the g
                # chunks (uninitialized SBUF could hold NaN; 0*NaN = NaN)
                if "hoist_packs" in ABLATE:
                    pka, pkb, pkc = hoisted_packs[(b, h)]
                    g0 = gfpool.tile([GC, LFULL], f16, tag="gf", name="g0")
                    g1 = gfpool.tile([GC, LFULL], f16, tag="gf", name="g1")
                    nc.vector.tensor_mul(g0[:, :], pka[:, :], pkb[:, :])
                    nc.vector.tensor_mul(g1[:, :], pka[:, :], pkc[:, :])
                    return (g0, g1)
                pk1 = packpool.tile([64, LFULL], f16, tag="pk1", name="pk1")
                pk2 = packpool.tile([64, LFULL], f16, tag="pk2", name="pk2")
                pk3 = packpool.tile([64, LFULL], f16, tag="pk3", name="pk3")
                nc.sync.dma_start(pk1[:, :], x_t[8:72, :])
                nc.sync.dma_start(pk2[0:56, :], x_t[16:72, :])
                nc.sync.dma_start(pk2[56:64, :], x_t[0:8, :])
                nc.sync.dma_start(pk3[0:48, :], x_t[24:72, :])
                nc.sync.dma_start(pk3[48:64, :], x_t[0:16, :])
                g0 = gfpool.tile([GC, LFULL], f16, tag="gf", name="g0")
                g1 = gfpool.tile([GC, LFULL], f16, tag="gf", name="g1")
                nc.vector.tensor_mul(g0[0:64, :], x_t[0:64, :], pk1[:, :])
                nc.vector.tensor_mul(g0[64:128, :], x_t[0:64, :], pk2[:, :])
                if D0_SPLIT and h == 0:
                    nc.scalar.square(g1[0:64, :], x_t[0:64, :])
                else:
                    d0eng = nc.gpsimd if D0_GPS else nc.vector
                    d0eng.tensor_mul(g1[0:64, :], x_t[0:64, :], x_t[0:64, :])
                nc.vector.tensor_mul(g1[64:128, :], x_t[0:64, :], pk3[:, :])
                return (g0, g1)

            # seed with each engine's fixed load (ACT: 16 sel-squares
            # ~1.3us each; DVE: 8 fused product ops ~2.56us each) so the
            # greedy copy placement balances total engine time
            eng_busy = {"act": 16 * 1.3, "dve": 8 * 2.56}

            def out_copy(o_view, ps_view, cost_act, cost_dve):
                if eng_busy["act"] + cost_act < eng_busy["dve"] + cost_dve:
                    nc.scalar.copy(o_view, ps_view)
                    eng_busy["act"] += cost_act
                else:
                    nc.vector.tensor_copy(o_view, ps_view)
                    eng_busy["dve"] += cost_dve

            def do_mains(st):
                b, ptiles, gchunks, selg = st
                if "no_mains" in ABLATE:
                    return
                ps_o = popool.tile([O, 2, 512], f32, tag="ps_o", name="ps_o")
                for h in range(2):
                    for ch in range(3):
                        kk = h * 3 + ch
                        for j, (ho0, nr) in enumerate(ptiles):
                            lt = nr * 64
                            if ch == 2:
                                mov = selg[j][:, h, :lt]
                            else:
                                c0 = ho0 * 64
                                mov = gchunks[h][ch][:, c0 : c0 + lt]
                            nc.tensor.matmul(
                                ps_o[:, j, :lt],
                                w_r[:, kk, :],
                                mov,
                                start=(kk == 0),
                                stop=(kk == 5),
                            )
                o_t = opool.tile([O, 2, 8 * WO], f32, tag="o", name="o_t")
                nr0, nr1 = ptiles[0][1], ptiles[1][1]
                if nr0 == nr1:
                    ps_view = ps_o[:].rearrange("o j (r w) -> o j r w", w=64)
                    o_view = o_t[:, :, : nr0 * WO].rearrange(
                        "o j (r w) -> o j r w", w=WO
                    )
                    out_copy(o_view, ps_view[:, :, :nr0, :WO], 1.92, 1.59)
                else:
                    for j, (ho0, nr) in enumerate(ptiles):
                        ps_view = ps_o[:, j, : nr * 64].rearrange(
                            "o (r w) -> o r w", w=64
                        )
                        o_view = o_t[:, j, : nr * WO].rearrange(
                            "o (r w) -> o r w", w=WO
                        )
                        out_copy(o_view, ps_view[:, :, :WO], 1.1, 0.95)
                # paired store: rows ho0..ho0+nr are contiguous in o_d
                # (tile j=1's rows sit at l = 8*WO in the flattened o_t)
                ho0 = ptiles[0][0]
                nr = nr0 + nr1
                nc.sync.dma_start(
                    o_d[b, :, ho0 : ho0 + nr, :],
                    o_t[:].rearrange("o j l -> o (j l)")[:, : nr * WO],
                )

            def warmup():
                for i in range(12):
                    ps_w = popool.tile([O, 2, 512], f32, tag="ps_o", name="warm")
                    nc.tensor.matmul(
                        ps_w[:, 0, :512], a_r[:, :128], xr_all[0][0][:, :512],
                        start=True, stop=True,
                    )

            def body(it=None, unroll=1):
                pending = []
                for b in range(B_LOC):
                    if "const_g" in ABLATE:
                        gchunks = [gconst, gconst]
                    else:
                        gchunks = [make_products(b, h) for h in range(2)]
                    for ptiles in TILE_PAIRS:
                        if "const_selg" in ABLATE:
                            pending.append((b, ptiles, gchunks,
                                            [sconst, sconst]))
                            if len(pending) > skew:
                                do_mains(pending.pop(0))
                            continue
                        selg = []
                        for j, (ho0, nr) in enumerate(ptiles):
                            lt = nr * 64
                            c0 = ho0 * 64
                            ps_s = pspool.tile(
                                [GC, 2, 512], f32, tag="ps_s", name="ps_s"
                            )
                            for h in range(2):
                                nc.tensor.matmul(
                                    ps_s[:, h, :lt],
                                    a_r[:],
                                    xr_all[b][h][:, c0 : c0 + lt],
                                    start=True,
                                    stop=True,
                                )
                            g_t = gspool.tile([GC, 2, 512], f16, tag="gs",
                                              name="g_t")
                            nc.scalar.square(g_t[:, :, :lt], ps_s[:, :, :lt])
                            selg.append(g_t)
                        pending.append((b, ptiles, gchunks, selg))
                        if len(pending) > skew:
                            do_mains(pending.pop(0))
                for st in pending:
                    do_mains(st)

            warmup()
            if static_reps > 1:
                for _ in range(static_reps):
                    body()
            elif reps == 1:
                body()
            else:
                hint = (
                    mybir.EngineType.PE,
                    mybir.EngineType.Activation,
                    mybir.EngineType.DVE,
                    mybir.EngineType.SP,
                    mybir.EngineType.Pool,
                )
                with tc.For_i(0, reps, 1, hint_engines=hint) as _it:
                    body()
    if DEDUP_LDW:
        build_nc.last_dedup = _dedup_ldweights(nc)
    nc.compile()
    return nc


@functools.lru_cache(maxsize=1)
def _cached_nc():
    return build_nc()


def make_in_maps(x: np.ndarray, W: np.ndarray) -> list:
    x = np.asarray(x, dtype=np.float32)
    W = np.asarray(W, dtype=np.float32)
    AselT, W2T = _build_consts(W)
    x_r = x.astype(np.float16)
    return [
        {
            "x_loc": np.ascontiguousarray(x_r[k * B_LOC : (k + 1) * B_LOC]),
            "aselT": AselT,
            "w2T": W2T,
        }
        for k in range(N_CORES)
    ]


def kernel(x: np.ndarray, W: np.ndarray, _trace: bool = False):
    nc = _cached_nc()
    in_maps = make_in_maps(x, W)
    try:
        r = run_bass_kernel_spmd(
            nc, in_maps, core_ids=list(range(N_CORES)), trace=_trace
        )
    except Exception:
        # transient NRT_EXEC_UNIT_UNRECOVERABLE has been observed once on
        # this fabric; a fresh attempt recovers
        r = run_bass_kernel_spmd(
            nc, in_maps, core_ids=list(range(N_CORES)), trace=_trace
        )
    out = np.concatenate([m["out_loc"] for m in r.results], axis=0)
    if _trace:
        kernel.last_result = r
    return out


if __name__ == "__main__":
    rng = np.random.default_rng(0)
    x = rng.standard_normal((B, C, H, WIDTH), dtype=np.float32)
    W = rng.standard_normal((O, C * 81), dtype=np.float32)
    out = kernel(x, W)
    print("out shape", out.shape, out.dtype)


# revision 26
# speedup vs baseline: 1.2033x; 1.2033x over previous
"""Trainium2 Bass kernel for nn_Fast2Order_DE_Conv (hybrid basis v4).

Math: out[b,o,ho,wo] = sum_{c,i,j} W[o, c*81+i*9+j] * p_i * p_j with
p_i = x[b, c, ho+di, wo+dj] (i = di*3+dj, 3x3 unfold of a 16-channel 64x64
image; output 62x62).

Quadratic features are evaluated in a HYBRID basis that splits work across
every engine (per channel, pairs keyed by index distance d = j - i):
  d in {1,2,3}: direct products x_i * x_j          -> DVE tensor_tensor
  d = 0, i<8  : squares x_i^2                      -> GPSIMD tensor_tensor
  d in {4..8} + x_8^2: (x_i+x_j)^2 squares basis   -> PE selection matmul
                                                      + ACT square
The basis change is folded into W on the host.  Per 512-column spatial
tile the PE runs only 2 selection + 6 main matmuls (vs 12 in a pure
squares-basis kernel); the d<=3 product features are computed at full
3968-column width, with the shifted second operands staged by SBUF->SBUF
DMA into partition-aligned pack tiles (DVE ops cannot read unaligned
partition bases).

Sharding: data-parallel over batch, 2 batches per core on 8 cores.
"""

import functools

import numpy as np

import concourse.bacc as bacc
import concourse.mybir as mybir
from concourse.tile import TileContext
from concourse.bass_utils import run_bass_kernel_spmd

B, C, H, WIDTH = 16, 16, 64, 64
O = 128
HO = WO = 62
N_CORES = 8
B_LOC = B // N_CORES
TILE_PAIRS = [((0, 8), (8, 8)), ((16, 8), (24, 8)),
              ((32, 8), (40, 8)), ((48, 8), (56, 6))]
GC = 128
SEL_PAIRS = [(i, i + d) for d in range(4, 9) for i in range(0, 9 - d)]  # 15
DEDUP_LDW = True
D0_GPS = False
ABLATE = {"hoist_packs"}
D0_SPLIT = False
X_BUFS = 1
GF_BUFS = 8
GS_BUFS = 6
O_BUFS = 2
PK_BUFS = 1
PSS_BUFS = 2
PSO_BUFS = 2


def _build_consts(Wf: np.ndarray):
    """W (128, 1296) -> (AselT [72, 128] f16, W2T [768, 128] f16).

    Per half (8 channels), 384 rows = 3 chunks of 128:
      chunk0: rows i*8+cl    (i 0..7) -> product (i, i+1)   [64]
              rows 64+i*8+cl (i 0..6) -> product (i, i+2)   [56]
      chunk1: rows i*8+cl    (i 0..7) -> x_i^2              [64]
              rows 64+i*8+cl (i 0..5) -> product (i, i+3)   [48]
      chunk2: sel (x_i+x_j)^2 for (i,j) in SEL_PAIRS [120], x_8^2 [8]
    """
    Wt = np.asarray(Wf, dtype=np.float64).reshape(O, C, 9, 9)
    Wsym = Wt + Wt.transpose(0, 1, 3, 2)
    W2 = np.zeros((O, 2, 3, 128))
    AselT = np.zeros((72, 128))
    for h in range(2):
        for cl in range(8):
            c = h * 8 + cl
            for i in range(8):
                W2[:, h, 0, i * 8 + cl] = Wsym[:, c, i, i + 1]
            for i in range(7):
                W2[:, h, 0, 64 + i * 8 + cl] = Wsym[:, c, i, i + 2]
            for i in range(8):
                W2[:, h, 1, i * 8 + cl] = Wt[:, c, i, i] - 0.5 * sum(
                    Wsym[:, c, a, b] for (a, b) in SEL_PAIRS if i in (a, b)
                )
            for i in range(6):
                W2[:, h, 1, 64 + i * 8 + cl] = Wsym[:, c, i, i + 3]
            for pi, (i, j) in enumerate(SEL_PAIRS):
                W2[:, h, 2, pi * 8 + cl] = 0.5 * Wsym[:, c, i, j]
            W2[:, h, 2, 120 + cl] = Wt[:, c, 8, 8] - 0.5 * sum(
                Wsym[:, c, a, b] for (a, b) in SEL_PAIRS if 8 in (a, b)
            )
    for cl in range(8):
        for pi, (i, j) in enumerate(SEL_PAIRS):
            AselT[i * 8 + cl, pi * 8 + cl] = 1.0
            AselT[j * 8 + cl, pi * 8 + cl] += 1.0
        AselT[64 + cl, 120 + cl] = 1.0
    W2T = np.ascontiguousarray(
        W2.transpose(1, 2, 3, 0).reshape(768, O)
    ).astype(np.float16)
    return AselT.astype(np.float16), W2T


def _x_window_ap(x_d, b: int, h: int, di: int, lt_load: int):
    """Source AP for one di of the unfold load: (dj, c, l) nesting matching
    target partitions (di*3+dj)*8 + c, free dim = padded l' = ho*64+wo."""
    ap = x_d[b, h * 8 : (h + 1) * 8, di, 0:3].unsqueeze(-1)
    v = ap.ap
    v[0] = [1, 3]
    v[1] = [H * WIDTH, 8]
    v[2] = [1, lt_load]
    return ap


def _dedup_ldweights(nc) -> int:
    """Remove InstLdweights whose weights AP matches the previous load in
    the same block (the PE array retains its stationary operand)."""
    removed = 0
    for fn in nc.m.functions:
        for blk in fn.blocks:
            insts = list(blk.instructions)
            cur_key = None
            drop = []
            for idx, inst in enumerate(insts):
                tn = type(inst).__name__
                if tn == "InstLdweights":
                    ap = inst.ins[0]
                    key = (
                        ap.memref, ap.offset, str(ap.ap), str(ap.dtype),
                        str(inst.perf_mode), str(inst.is_transpose),
                        str(inst.tile_position), str(inst.tile_size),
                    )
                    si = inst.sync_info
                    clean = si is None or (not si.on_wait and not si.on_update)
                    if key == cur_key and clean:
                        drop.append(idx)
                        removed += 1
                        continue
                    cur_key = key
                elif tn == "InstMatmult":
                    if getattr(inst, "is_transpose", None):
                        cur_key = None
            for idx in reversed(drop):
                del blk.instructions[idx]
    return removed


def build_nc(reps: int = 1, skew: int = 2, static_reps: int = 1):
    """Build the per-core program.  reps>1 wraps the body in an on-chip
    loop (For_i) for device-time measurement; skew is the pipeline depth
    in tile-pairs between selection/squares and main matmuls."""
    f32, f16 = mybir.dt.float32, mybir.dt.float16
    nc = bacc.Bacc("TRN2", target_bir_lowering=False)
    x_d = nc.dram_tensor("x_loc", [B_LOC, C, H, WIDTH], f16, kind="ExternalInput")
    a_d = nc.dram_tensor("aselT", [72, GC], f16, kind="ExternalInput")
    w_d = nc.dram_tensor("w2T", [6 * GC, O], f16, kind="ExternalInput")
    o_d = nc.dram_tensor("out_loc", [B_LOC, O, HO, WO], f32, kind="ExternalOutput")

    LFULL = HO * 64  # 3968

    with TileContext(nc) as tc:
        with (
            tc.tile_pool(name="const", bufs=1) as cpool,
            tc.tile_pool(name="xin", bufs=X_BUFS) as xpool,
            tc.tile_pool(name="packs", bufs=PK_BUFS) as packpool,
            tc.tile_pool(name="gfull", bufs=GF_BUFS) as gfpool,
            tc.tile_pool(name="gsel", bufs=GS_BUFS) as gspool,
            tc.tile_pool(name="obuf", bufs=O_BUFS) as opool,
            tc.tile_pool(name="ps_sel", bufs=PSS_BUFS, space="PSUM") as pspool,
            tc.tile_pool(name="ps_out", bufs=PSO_BUFS, space="PSUM") as popool,
        ):
            a_r = cpool.tile([72, GC], f16, tag="a_r")
            nc.sync.dma_start(a_r[:], a_d[:])

            def load_x(x_t, b, h, col0, col1):
                for di in range(3):
                    hi = min(col1, H * WIDTH - di * 64 - 2)
                    if hi > col0:
                        ap = _x_window_ap(x_d, b, h, di, hi - col0)
                        ap.offset += col0
                        nc.sync.dma_start(
                            x_t[di * 24 : (di + 1) * 24, col0:hi], ap
                        )
                    if hi < col1:
                        nc.sync.dma_start(
                            x_t[di * 24 : (di + 1) * 24, hi:col1],
                            _x_window_ap(x_d, b, h, 0, col1 - hi),
                        )

            xr_all = [
                [
                    xpool.tile([72, LFULL], f16, tag=f"x{b}{h}", name=f"x{b}{h}")
                    for h in range(2)
                ]
                for b in range(B_LOC)
            ]
            for h in range(2):
                load_x(xr_all[0][h], 0, h, 0, 1024)
            w_r = cpool.tile([GC, 6, O], f16, tag="w_r")
            nc.sync.dma_start(w_r[:], w_d[:].rearrange("(k p) o -> p k o", p=GC))
            for h in range(2):
                load_x(xr_all[0][h], 0, h, 1024, LFULL)
            for b in range(1, B_LOC):
                for h in range(2):
                    load_x(xr_all[b][h], b, h, 0, LFULL)

            gconst = None
            sconst = None
            if "const_g" in ABLATE or "const_selg" in ABLATE:
                if "const_g" in ABLATE:
                    gconst = [cpool.tile([GC, LFULL], f16, tag=f"gc{i}",
                                         name=f"gc{i}") for i in range(2)]
                    for t in gconst:
                        nc.vector.memset(t[:], 0.01)
                if "const_selg" in ABLATE:
                    sconst = cpool.tile([GC, 2, 512], f16, tag="sc")
                    nc.vector.memset(sconst[:], 0.01)
            if "no_mains" in ABLATE:
                o_zero = cpool.tile([O, 8 * WO], f32, tag="o_zero")
                nc.vector.memset(o_zero[:], 0.0)
                for b in range(B_LOC):
                    for ho0, nr in [(i * 8, 8) for i in range(7)] + [(56, 6)]:
                        nc.sync.dma_start(
                            o_d[b, :, ho0 : ho0 + nr, :], o_zero[:, : nr * WO]
                        )

            hoisted_packs = {}
            if "hoist_packs" in ABLATE:
                # 128-row operand packs: one DVE op per g chunk.
                # APack = [x_t[0:64]; x_t[0:64]] is the A side of both
                # chunks; BPack0/1 carry the shifted B operands (+ valid
                # junk in the zero-weight pad rows).
                for b in range(B_LOC):
                    for h in range(2):
                        x_t = xr_all[b][h]
                        pks = (
                            cpool.tile([128, LFULL], f16, tag=f"hpa{b}{h}",
                                       name=f"hpa{b}{h}"),
                            cpool.tile([128, LFULL], f16, tag=f"hpb{b}{h}",
                                       name=f"hpb{b}{h}"),
                            cpool.tile([128, LFULL], f16, tag=f"hpc{b}{h}",
                                       name=f"hpc{b}{h}"),
                        )
                        nc.sync.dma_start(pks[0][0:64, :], x_t[0:64, :])
                        nc.sync.dma_start(pks[0][64:128, :], x_t[0:64, :])
                        nc.sync.dma_start(pks[1][0:64, :], x_t[8:72, :])
                        nc.sync.dma_start(pks[1][64:120, :], x_t[16:72, :])
                        nc.sync.dma_start(pks[1][120:128, :], x_t[0:8, :])
                        nc.sync.dma_start(pks[2][0:64, :], x_t[0:64, :])
                        nc.sync.dma_start(pks[2][64:112, :], x_t[24:72, :])
                        nc.sync.dma_start(pks[2][112:128, :], x_t[0:16, :])
                        hoisted_packs[(b, h)] = pks

            def make_products(b, h):
                """Full-width product chunks 0/1 for (batch, half)."""
                x_t = xr_all[b][h]
                # base-0 staged copies of the shifted B operands (TensorTensor
                # requires equal base partitions for both SBUF inputs)
                # pk2/pk3 are widened to 64 rows with valid junk so the
                # product ops also fill the zero-weight pad rows of the g
                # chunks (uninitialized SBUF could hold NaN; 0*NaN = NaN)
                if "hoist_packs" in ABLATE:
                    pka, pkb, pkc = hoisted_packs[(b, h)]
                    g0 = gfpool.tile([GC, LFULL], f16, tag="gf", name="g0")
                    g1 = gfpool.tile([GC, LFULL], f16, tag="gf", name="g1")
                    nc.vector.tensor_mul(g0[:, :], pka[:, :], pkb[:, :])
                    nc.vector.tensor_mul(g1[:, :], pka[:, :], pkc[:, :])
                    return (g0, g1)
                pk1 = packpool.tile([64, LFULL], f16, tag="pk1", name="pk1")
                pk2 = packpool.tile([64, LFULL], f16, tag="pk2", name="pk2")
                pk3 = packpool.tile([64, LFULL], f16, tag="pk3", name="pk3")
                nc.sync.dma_start(pk1[:, :], x_t[8:72, :])
                nc.sync.dma_start(pk2[0:56, :], x_t[16:72, :])
                nc.sync.dma_start(pk2[56:64, :], x_t[0:8, :])
                nc.sync.dma_start(pk3[0:48, :], x_t[24:72, :])
                nc.sync.dma_start(pk3[48:64, :], x_t[0:16, :])
                g0 = gfpool.tile([GC, LFULL], f16, tag="gf", name="g0")
                g1 = gfpool.tile([GC, LFULL], f16, tag="gf", name="g1")
                nc.vector.tensor_mul(g0[0:64, :], x_t[0:64, :], pk1[:, :])
                nc.vector.tensor_mul(g0[64:128, :], x_t[0:64, :], pk2[:, :])
                if D0_SPLIT and h == 0:
                    nc.scalar.square(g1[0:64, :], x_t[0:64, :])
                else:
                    d0eng = nc.gpsimd if D0_GPS else nc.vector
                    d0eng.tensor_mul(g1[0:64, :], x_t[0:64, :], x_t[0:64, :])
                nc.vector.tensor_mul(g1[64:128, :], x_t[0:64, :], pk3[:, :])
                return (g0, g1)

            # seed with each engine's fixed load (ACT: 16 sel-squares
            # ~1.3us each; DVE: 8 fused product ops ~2.56us each) so the
            # greedy copy placement balances total engine time
            eng_busy = {"act": 16 * 1.3, "dve": 8 * 2.56}

            def out_copy(o_view, ps_view, cost_act, cost_dve):
                if eng_busy["act"] + cost_act < eng_busy["dve"] + cost_dve:
                    nc.scalar.copy(o_view, ps_view)
                    eng_busy["act"] += cost_act
                else:
                    nc.vector.tensor_copy(o_view, ps_view)
                    eng_busy["dve"] += cost_dve

            def do_mains(st):
                b, ptiles, gchunks, selg = st
                if "no_mains" in ABLATE:
                    return
                ps_o = popool.tile([O, 2, 512], f32, tag="ps_o", name="ps_o")
                for h in range(2):
                    for ch in range(3):
                        kk = h * 3 + ch
                        for j, (ho0, nr) in enumerate(ptiles):
                            lt = nr * 64
                            if ch == 2:
                                mov = selg[j][:, h, :lt]
                            else:
                                c0 = ho0 * 64
                                mov = gchunks[h][ch][:, c0 : c0 + lt]
                            nc.tensor.matmul(
                                ps_o[:, j, :lt],
                                w_r[:, kk, :],
                                mov,
                                start=(kk == 0),
                                stop=(kk == 5),
                            )
                o_t = opool.tile([O, 2, 8 * WO], f32, tag="o", name="o_t")
                nr0, nr1 = ptiles[0][1], ptiles[1][1]
                if nr0 == nr1:
                    ps_view = ps_o[:].rearrange("o j (r w) -> o j r w", w=64)
                    o_view = o_t[:, :, : nr0 * WO].rearrange(
                        "o j (r w) -> o j r w", w=WO
                    )
                    out_copy(o_view, ps_view[:, :, :nr0, :WO], 1.92, 1.59)
                else:
                    for j, (ho0, nr) in enumerate(ptiles):
                        ps_view = ps_o[:, j, : nr * 64].rearrange(
                            "o (r w) -> o r w", w=64
                        )
                        o_view = o_t[:, j, : nr * WO].rearrange(
                            "o (r w) -> o r w", w=WO
                        )
                        out_copy(o_view, ps_view[:, :, :WO], 1.1, 0.95)
                # paired store: rows ho0..ho0+nr are contiguous in o_d
                # (tile j=1's rows sit at l = 8*WO in the flattened o_t)
                ho0 = ptiles[0][0]
                nr = nr0 + nr1
                nc.sync.dma_start(
                    o_d[b, :, ho0 : ho0 + nr, :],
                    o_t[:].rearrange("o j l -> o (j l)")[:, : nr * WO],
                )

            def warmup():
                for i in range(12):
                    ps_w = popool.tile([O, 2, 512], f32, tag="ps_o", name="warm")
                    nc.tensor.matmul(
                        ps_w[:, 0, :512], a_r[:, :128], xr_all[0][0][:, :512],
                        start=True, stop=True,
                    )

            def body(it=None, unroll=1):
                pending = []
                for b in range(B_LOC):
                    if "const_g" in ABLATE:
                        gchunks = [gconst, gconst]
                    else:
                        gchunks = [make_products(b, h) for h in range(2)]
                    for ptiles in TILE_PAIRS:
                        if "const_selg" in ABLATE:
                            pending.append((b, ptiles, gchunks,
                                            [sconst, sconst]))
                            if len(pending) > skew:
                                do_mains(pending.pop(0))
                            continue
                        selg = []
                        for j, (ho0, nr) in enumerate(ptiles):
                            lt = nr * 64
                            c0 = ho0 * 64
                            ps_s = pspool.tile(
                                [GC, 2, 512], f32, tag="ps_s", name="ps_s"
                            )
                            for h in range(2):
                                nc.tensor.matmul(
                                    ps_s[:, h, :lt],
                                    a_r[:],
                                    xr_all[b][h][:, c0 : c0 + lt],
                                    start=True,
                                    stop=True,
                                )
                            g_t = gspool.tile([GC, 2, 512], f16, tag="gs",
                                              name="g_t")
                            nc.scalar.square(g_t[:, :, :lt], ps_s[:, :, :lt])
                            selg.append(g_t)
                        pending.append((b, ptiles, gchunks, selg))
                        if len(pending) > skew:
                            do_mains(pending.pop(0))
                for st in pending:
                    do_mains(st)

            warmup()
            if static_reps > 1:
                for _ in range(static_reps):
                    body()
            elif reps == 1:
                body()
            else:
                hint = (
                    mybir.EngineType.PE,
                    mybir.EngineType.Activation,
                    mybir.EngineType.DVE,
                    mybir.EngineType.SP,
                    mybir.EngineType.Pool,
                )
                with tc.For_i(0, reps, 1, hint_engines=hint) as _it:
                    body()
    if DEDUP_LDW:
        build_nc.last_dedup = _dedup_ldweights(nc)
    nc.compile()
    return nc


@functools.lru_cache(maxsize=1)
def _cached_nc():
    return build_nc()


def make_in_maps(x: np.ndarray, W: np.ndarray) -> list:
    x = np.asarray(x, dtype=np.float32)
    W = np.asarray(W, dtype=np.float32)
    AselT, W2T = _build_consts(W)
    x_r = x.astype(np.float16)
    return [
        {
            "x_loc": np.ascontiguousarray(x_r[k * B_LOC : (k + 1) * B_LOC]),
            "aselT": AselT,
            "w2T": W2T,
        }
        for k in range(N_CORES)
    ]


def kernel(x: np.ndarray, W: np.ndarray, _trace: bool = False):
    nc = _cached_nc()
    in_maps = make_in_maps(x, W)
    try:
        r = run_bass_kernel_spmd(
            nc, in_maps, core_ids=list(range(N_CORES)), trace=_trace
        )
    except Exception:
        # transient NRT_EXEC_UNIT_UNRECOVERABLE has been observed once on
        # this fabric; a fresh attempt recovers
        r = run_bass_kernel_spmd(
            nc, in_maps, core_ids=list(range(N_CORES)), trace=_trace
        )
    out = np.concatenate([m["out_loc"] for m in r.results], axis=0)
    if _trace:
        kernel.last_result = r
    return out


if __name__ == "__main__":
    rng = np.random.default_rng(0)
    x = rng.standard_normal((B, C, H, WIDTH), dtype=np.float32)
    W = rng.standard_normal((O, C * 81), dtype=np.float32)
    out = kernel(x, W)
    print("out shape", out.shape, out.dtype)
